# revision 1
# baseline (speedup 1.0000x reference)
"""EquiConv (DeepH-E3) Trainium2 kernel — 8-core data-parallel over edges.

Strategy (channel-major on device):
  - Host folds all per-channel weights/constants into matmul weight
    matrices, shards edges across 8 cores, pads to a multiple of 512 and
    transposes edge tensors to channel-major [C, E].
  - Per-edge scalars (x2s, x2v_i) are shipped pre-replicated across 128
    partitions (host-side layout transform), so device tiles need only
    DVE multiplies to pre-scale activations; all tensor-product paths
    become accumulating float32r matmuls into PSUM, the Gate uses
    Silu/Tanh (one ACT table set; sigmoid(g)*w computed via a fused
    scalar_tensor_tensor from tanh(g/2)), and the e3ElementWise multiply
    is fused into the output elementwise ops. DMA issue is spread over
    the Sync and GpSimd descriptor queues.
  - Host transposes the [320, E] channel-major output back.

Self-contained: hardcodes shapes from the problem spec; no file reads.
"""
import os
import sys

import numpy as np

# ---------------------------------------------------------------- constants
E_FULL = 200000
N_CORES = 8
E_CORE = E_FULL // N_CORES      # 25000
NT = 512                        # edges per tile
T_TILES = 49                    # tiles per core
E_PAD = NT * T_TILES            # 25088
MUL_S = 128
MUL_V = 64

INV_S = 1.0 / np.sqrt(MUL_S)
INV_V = 1.0 / np.sqrt(MUL_V)
SQ2 = 1.0 / np.sqrt(2.0)
SQ3 = 1.0 / np.sqrt(3.0)

_REPO_CANDIDATES = (
    "/opt/trn_rl_repo",
    "/root/.axon_site/_ro/trn_rl_repo",
)


def _ensure_repo_on_path():
    try:
        import concourse.bass  # noqa: F401
        return
    except ImportError:
        pass
    for p in _REPO_CANDIDATES:
        if os.path.isdir(p) and p not in sys.path:
            sys.path.insert(0, p)
    import concourse.bass  # noqa: F401


_CACHE = {}


def _build_nc():
    """Build + compile the per-core Bass program (cached)."""
    if "nc" in _CACHE:
        return _CACHE["nc"]
    _ensure_repo_on_path()
    import concourse.mybir as mybir
    import concourse.tile as tile
    from concourse import bacc

    F32 = mybir.dt.float32
    F32R = mybir.dt.float32r
    MULT = mybir.AluOpType.mult
    ADD = mybir.AluOpType.add
    AF = mybir.ActivationFunctionType

    nc = bacc.Bacc(trn_type="TRN2", target_bir_lowering=False, debug=False,
                   num_devices=N_CORES)

    # DRAM inputs (per-core shard, channel-major) --------------------------
    d_x1s = nc.dram_tensor("x1s_t", [128, E_PAD], F32R, kind="ExternalInput")
    d_x1v = nc.dram_tensor("x1v_t", [192, E_PAD], F32R, kind="ExternalInput")
    d_rs = nc.dram_tensor("r_s", [128, E_PAD], F32R, kind="ExternalInput")
    d_rv0 = nc.dram_tensor("r_v0", [128, E_PAD], F32R, kind="ExternalInput")
    d_rv1 = nc.dram_tensor("r_v1", [128, E_PAD], F32R, kind="ExternalInput")
    d_rv2 = nc.dram_tensor("r_v2", [128, E_PAD], F32R, kind="ExternalInput")
    d_rv01 = nc.dram_tensor("r_v01", [128, E_PAD], F32R,
                            kind="ExternalInput")
    d_fw = nc.dram_tensor("fw_t", [128, E_PAD], F32R, kind="ExternalInput")
    # folded weights ([K, M] layouts, ready as lhsT)
    d_wa0 = nc.dram_tensor("wa0", [128, 128], F32R, kind="ExternalInput")
    d_wa1 = nc.dram_tensor("wa1", [128, 64], F32R, kind="ExternalInput")
    d_wp2 = nc.dram_tensor("wp2", [128, 64], F32R, kind="ExternalInput")
    d_wb4s = nc.dram_tensor("wb4s", [128, 128], F32R, kind="ExternalInput")
    d_wb4b = nc.dram_tensor("wb4b", [64, 128], F32R, kind="ExternalInput")
    d_wb5s = nc.dram_tensor("wb5s", [128, 64], F32R, kind="ExternalInput")
    d_wb5b = nc.dram_tensor("wb5b", [64, 64], F32R, kind="ExternalInput")
    d_wc = nc.dram_tensor("wc", [64, 64], F32R, kind="ExternalInput")
    d_fc0 = nc.dram_tensor("fc0", [128, 64], F32R, kind="ExternalInput")
    d_fc1 = nc.dram_tensor("fc1", [64, 64], F32R, kind="ExternalInput")
    d_fc2a = nc.dram_tensor("fc2a", [64, 128], F32R, kind="ExternalInput")
    d_fc2b = nc.dram_tensor("fc2b", [64, 64], F32R, kind="ExternalInput")
    d_b0 = nc.dram_tensor("b0c", [64, 1], F32, kind="ExternalInput")
    d_b1 = nc.dram_tensor("b1c", [64, 1], F32, kind="ExternalInput")
    d_b2a = nc.dram_tensor("b2a", [128, 1], F32, kind="ExternalInput")
    d_b2b = nc.dram_tensor("b2v", [64, 1], F32, kind="ExternalInput")

    d_out = nc.dram_tensor("out_t", [320, E_PAD], F32, kind="ExternalOutput")

    with tile.TileContext(nc) as tc:
        with tc.tile_pool(name="const", bufs=1) as cp, \
             tc.tile_pool(name="io", bufs=4) as io, \
             tc.tile_pool(name="work", bufs=2) as wk, \
             tc.tile_pool(name="ps", bufs=1, space="PSUM") as ps:

            # constants into SBUF once
            def const(d, shape, dtype=F32R, name=None):
                t = cp.tile(shape, dtype, name=name or d.name + "_sb")
                nc.sync.dma_start(t, d.ap())
                return t

            w_wa0 = const(d_wa0, [128, 128])
            w_wa1 = const(d_wa1, [128, 64])
            w_wp2 = const(d_wp2, [128, 64])
            w_wb4s = const(d_wb4s, [128, 128])
            w_wb4b = const(d_wb4b, [64, 128])
            w_wb5s = const(d_wb5s, [128, 64])
            w_wb5b = const(d_wb5b, [64, 64])
            w_wc = const(d_wc, [64, 64])
            w_fc0 = const(d_fc0, [128, 64])
            w_fc1 = const(d_fc1, [64, 64])
            w_fc2a = const(d_fc2a, [64, 128])
            w_fc2b = const(d_fc2b, [64, 64])
            c_b0 = const(d_b0, [64, 1], F32)
            c_b1 = const(d_b1, [64, 1], F32)
            c_b2a = const(d_b2a, [128, 1], F32)
            c_b2b = const(d_b2b, [64, 1], F32)
            # WC copy living at partitions 64-127 for the row-offset matmul
            w_wc_f = cp.tile([128, 64], F32R)
            w_wc_hi = w_wc_f[64:128, :]
            nc.sync.dma_start(w_wc_hi, d_wc.ap())

            for t in range(T_TILES):
                sl = slice(t * NT, (t + 1) * NT)

                # ---- loads -------------------------------------------
                x1s = io.tile([128, NT], F32R)
                nc.sync.dma_start(x1s, d_x1s.ap()[:, sl])
                x1va = io.tile([128, NT], F32R)
                nc.sync.dma_start(x1va, d_x1v.ap()[0:128, sl])
                x1vc2 = io.tile([64, NT], F32R)
                nc.sync.dma_start(x1vc2, d_x1v.ap()[128:192, sl])
                rep_s = io.tile([128, NT], F32R)
                nc.gpsimd.dma_start(rep_s, d_rs.ap()[:, sl])
                rep_v0 = io.tile([128, NT], F32R)
                nc.gpsimd.dma_start(rep_v0, d_rv0.ap()[:, sl])
                rep_v1 = io.tile([128, NT], F32R)
                nc.gpsimd.dma_start(rep_v1, d_rv1.ap()[:, sl])
                rep_v2 = io.tile([128, NT], F32R)
                nc.gpsimd.dma_start(rep_v2, d_rv2.ap()[:, sl])
                rep_v01 = io.tile([128, NT], F32R)
                nc.gpsimd.dma_start(rep_v01, d_rv01.ap()[:, sl])
                fwt = io.tile([128, NT], F32R)
                nc.gpsimd.dma_start(fwt, d_fw.ap()[:, sl])

                # ---- radial MLP --------------------------------------
                h1 = ps.tile([64, NT], F32, tag="mlp")
                nc.tensor.matmul(h1, w_fc0, fwt, start=True, stop=True)
                h1s = wk.tile([64, NT], F32R)
                nc.scalar.activation(h1s, h1, AF.Silu, bias=c_b0)
                h2 = ps.tile([64, NT], F32, tag="mlp")
                nc.tensor.matmul(h2, w_fc1, h1s, start=True, stop=True)
                h2s = wk.tile([64, NT], F32R)
                nc.scalar.activation(h2s, h2, AF.Silu, bias=c_b1)
                wwa = ps.tile([128, NT], F32, tag="mlp")
                nc.tensor.matmul(wwa, w_fc2a, h2s, start=True, stop=True)
                wwa_s = wk.tile([128, NT], F32)
                nc.scalar.activation(wwa_s, wwa, AF.Identity, bias=c_b2a)
                wwb = ps.tile([64, NT], F32, tag="mlp")
                nc.tensor.matmul(wwb, w_fc2b, h2s, start=True, stop=True)
                wwb_s = wk.tile([64, NT], F32)
                # 0.5*(wwb + b2v): folds the sigmoid's 0.5 factor
                nc.scalar.activation(wwb_s, wwb, AF.Identity, bias=c_b2b,
                                     scale=0.5)

                # ---- prescales (ordered to unblock PE early) ---------
                x1s_s = wk.tile([128, NT], F32R, bufs=3)
                nc.vector.tensor_tensor(x1s_s, x1s, rep_s, MULT)
                x1s_v0 = wk.tile([128, NT], F32R, bufs=3)
                nc.vector.tensor_tensor(x1s_v0, x1s, rep_v0, MULT)
                x1s_v1 = wk.tile([128, NT], F32R, bufs=3)
                nc.vector.tensor_tensor(x1s_v1, x1s, rep_v1, MULT)
                x1s_v2 = wk.tile([128, NT], F32R, bufs=3)
                nc.vector.tensor_tensor(x1s_v2, x1s, rep_v2, MULT)
                xv_s01 = wk.tile([128, NT], F32R, bufs=3)
                nc.vector.tensor_tensor(xv_s01, x1va, rep_s, MULT)
                xv_p01 = wk.tile([128, NT], F32R, bufs=3)
                nc.vector.tensor_tensor(xv_p01, x1va, rep_v01, MULT)
                xv_s2 = wk.tile([64, NT], F32R, bufs=3)
                nc.vector.tensor_tensor(xv_s2, x1vc2, rep_s[0:64, :], MULT)
                xv_p2 = wk.tile([64, NT], F32R, bufs=3)
                nc.vector.tensor_tensor(xv_p2, x1vc2, rep_v2[0:64, :], MULT)

                # ---- tensor-product matmuls (consumption order) ------
                scal = ps.tile([128, NT], F32, tag="scal", bufs=2)
                gate = ps.tile([64, NT], F32, tag="gate", bufs=2)
                vec0 = ps.tile([64, NT], F32, tag="vec0")
                vec1 = ps.tile([64, NT], F32, tag="vec1")
                vec2 = ps.tile([64, NT], F32, tag="vec2")
                nc.tensor.matmul(scal, w_wa0, x1s_s, start=True, stop=False)
                nc.tensor.matmul(gate, w_wa1, x1s_s, start=True, stop=False)
                nc.tensor.matmul(vec0, w_wp2, x1s_v0, start=True, stop=False)
                nc.tensor.matmul(vec1, w_wp2, x1s_v1, start=True, stop=False)
                nc.tensor.matmul(vec2, w_wp2, x1s_v2, start=True, stop=False)
                nc.tensor.matmul(vec0, w_wc, xv_s01[0:64, :],
                                 start=False, stop=True)
                nc.tensor.matmul(vec1, w_wc_hi, xv_s01[64:128, :],
                                 start=False, stop=True,
                                 tile_position=(64, 0))
                nc.tensor.matmul(scal, w_wb4s, xv_p01, start=False, stop=False)
                nc.tensor.matmul(gate, w_wb5s, xv_p01, start=False, stop=False)
                nc.tensor.matmul(vec2, w_wc, xv_s2, start=False, stop=True)
                nc.tensor.matmul(scal, w_wb4b, xv_p2, start=False, stop=True)
                nc.tensor.matmul(gate, w_wb5b, xv_p2, start=False, stop=True)

                # ---- gate + e3ElementWise ----------------------------
                sc_silu = wk.tile([128, NT], F32)
                nc.scalar.activation(sc_silu, scal, AF.Silu)
                tgate = wk.tile([64, NT], F32)
                nc.scalar.activation(tgate, gate, AF.Tanh, scale=0.5)
                # sgw = (tanh(g/2)+1) * 0.5*(w_vec+b) = sigmoid(g)*w_vec
                sgw = wk.tile([64, NT], F32)
                nc.vector.scalar_tensor_tensor(sgw, tgate, 1.0, wwb_s,
                                               ADD, MULT)

                out_s = wk.tile([128, NT], F32)
                nc.vector.tensor_tensor(out_s, sc_silu, wwa_s, MULT)
                out0 = wk.tile([64, NT], F32)
                nc.vector.tensor_tensor(out0, vec0, sgw, MULT)
                out1 = wk.tile([64, NT], F32)
                nc.vector.tensor_tensor(out1, vec1, sgw, MULT)
                out2 = wk.tile([64, NT], F32)
                nc.vector.tensor_tensor(out2, vec2, sgw, MULT)

                # ---- stores ------------------------------------------
                nc.sync.dma_start(d_out.ap()[0:128, sl], out_s)
                nc.sync.dma_start(d_out.ap()[128:192, sl], out0)
                nc.sync.dma_start(d_out.ap()[192:256, sl], out1)
                nc.sync.dma_start(d_out.ap()[256:320, sl], out2)

    nc.compile()
    _CACHE["nc"] = nc
    return nc


def _fold_weights(inp):
    """Fold per-channel weights + constants into matmul matrices."""
    f = lambda k: np.asarray(inp[k], dtype=np.float32)
    w0f = f("w1_p0") * f("w2_p0")[None, :] * (INV_S * SQ2)
    w1f = f("w1_p1") * f("w2_p1")[None, :] * (INV_S * SQ2)
    w2f = f("w1_p2") * f("w2_p2")[None, :] * (INV_S * SQ2)
    w3f = f("w1_p3") * f("w2_p3")[None, :] * (INV_V * SQ2)
    w4f = f("w1_p4") * f("w2_p4")[None, :] * (INV_V * SQ3 * SQ2)
    w5f = f("w1_p5") * f("w2_p5")[None, :] * (INV_V * SQ3 * SQ2)
    fc2 = f("fc_w2")
    b2 = f("fc_b2")
    c = np.ascontiguousarray
    return {
        "wa0": c(w0f),
        "wa1": c(w1f),
        "wp2": c(w2f),
        "wb4s": c(np.concatenate([w4f, w4f], axis=0)),
        "wb4b": c(w4f),
        "wb5s": c(np.concatenate([w5f, w5f], axis=0)),
        "wb5b": c(w5f),
        "wc": c(w3f),
        "fc0": c(f("fc_w0")),
        "fc1": c(f("fc_w1")),
        "fc2a": c(fc2[:, :128]),
        "fc2b": c(fc2[:, 128:]),
        "b0c": c(f("fc_b0")[:, None]),
        "b1c": c(f("fc_b1")[:, None]),
        "b2a": c(b2[:128, None]),
        "b2v": c(b2[128:, None]),

    }


def _shard_inputs(inp):
    """Per-core channel-major shards (padded to E_PAD edges)."""
    fea_in1 = np.asarray(inp["fea_in1"], dtype=np.float32)
    fea_in2 = np.asarray(inp["fea_in2"], dtype=np.float32)
    fea_w = np.asarray(inp["fea_weight"], dtype=np.float32)
    shards = []
    for c in range(N_CORES):
        s = slice(c * E_CORE, (c + 1) * E_CORE)
        x1 = fea_in1[s]
        x2 = fea_in2[s]
        fw = fea_w[s]
        x1s_t = np.zeros((128, E_PAD), np.float32)
        x1s_t[:, :E_CORE] = x1[:, :128].T
        x1v_t = np.zeros((192, E_PAD), np.float32)
        x1v_t[:, :E_CORE] = (
            x1[:, 128:].reshape(E_CORE, 64, 3).transpose(2, 1, 0)
            .reshape(192, E_CORE))
        fw_t = np.zeros((128, E_PAD), np.float32)
        fw_t[:, :E_CORE] = fw.T
        x2p = np.zeros((E_PAD, 4), np.float32)
        x2p[:E_CORE] = x2

        def rep128(row):
            return np.ascontiguousarray(
                np.broadcast_to(row[None, :], (128, E_PAD)))

        r_v01 = np.empty((128, E_PAD), np.float32)
        r_v01[:64] = x2p[:, 1]
        r_v01[64:] = x2p[:, 2]
        shards.append({
            "x1s_t": np.ascontiguousarray(x1s_t),
            "x1v_t": np.ascontiguousarray(x1v_t),
            "fw_t": np.ascontiguousarray(fw_t),
            "r_s": rep128(x2p[:, 0]),
            "r_v0": rep128(x2p[:, 1]),
            "r_v1": rep128(x2p[:, 2]),
            "r_v2": rep128(x2p[:, 3]),
            "r_v01": r_v01,
        })
    return shards


def run(inputs, trace=False, trace_kwargs=None):
    """Run the kernel; returns (output [E,320] f32, BassKernelResults)."""
    _ensure_repo_on_path()
    from concourse import bass_utils

    nc = _build_nc()
    weights = _fold_weights(inputs)
    shards = _shard_inputs(inputs)
    in_maps = [{**weights, **sh} for sh in shards]

    kwargs = {}
    if trace:
        _install_ntff_hook()
        kwargs.update(trace=True, **(trace_kwargs or {}))
    res = bass_utils.run_bass_kernel_spmd(
        nc, in_maps, core_ids=list(range(N_CORES)), **kwargs)

    out = np.empty((E_FULL, 320), np.float32)
    for c in range(N_CORES):
        o = res.results[c]["out_t"][:, :E_CORE]          # [320, 25000]
        s = slice(c * E_CORE, (c + 1) * E_CORE)
        out[s, :128] = o[:128].T
        out[s, 128:] = (o[128:].reshape(3, 64, E_CORE)
                        .transpose(2, 1, 0).reshape(E_CORE, 192))
    return out, res


def _install_ntff_hook():
    """Shim the missing antenv.axon_hooks so trace=True works under axon."""
    import types
    import antenv
    from concourse import bass_utils
    if "antenv.axon_hooks" in sys.modules:
        return
    mod = types.ModuleType("antenv.axon_hooks")
    _h = [None]
    mod.set_axon_ntff_profile_hook = lambda h: _h.__setitem__(0, h)
    mod.get_axon_ntff_profile_hook = lambda: _h[0]
    sys.modules["antenv.axon_hooks"] = mod
    antenv.axon_hooks = mod
    from trn_agent_boot.trn_boot import _ntff_profile_via_ctypes
    mod.set_axon_ntff_profile_hook(
        _ntff_profile_via_ctypes("/opt/axon/libaxon_pjrt.so"))
    bass_utils.upload_artifacts = lambda tmpdir: tmpdir


def kernel(**inputs) -> np.ndarray:
    out, _ = run(inputs, trace=False)
    return out



# revision 11
# speedup vs baseline: 1.2223x; 1.2223x over previous
"""EquiConv (DeepH-E3) Trainium2 kernel — 8-core data-parallel over edges.

Strategy (channel-major, fp16 on-device, 3-stage software pipeline):
  - Host folds per-channel weights/constants into fp16 matmul matrices,
    shards edges across 8 cores, pads to 49x512 per core, and packs the
    six per-tile input row-blocks (x1s, x1v01, x1v2-dup, fea_weight and
    two packed replicated scalar blocks) into ONE interleaved DRAM
    tensor; the four full-height per-edge scalar replications come from
    a single partition-broadcast DMA of a tiny [4, E] tensor.
  - The P2 path uses zero-padded weight columns ([w2|0], [0|w2]) so both
    vector components accumulate into one diag(w3,w3)-anchored PSUM
    group with uniform tile config; gate and MLP-wb outputs are
    column-duplicated so the sigmoid gate covers all 128 partitions.
  - Emission is software-pipelined three tiles deep so every PE matmul's
    inputs are ready >=1 iteration ahead: the PE never stalls and ramps
    to its max p-state (~216ns per 512-col fp16 matmul vs 427ns).
  - All SBUF traffic is fp16 (2x DVE rate); PSUM stays f32.  ACT does
    the PSUM->SBUF crossings, Pool (GpSimd) one prescale + half the
    sigmoid chain, DVE the rest.
  - Output is written fp16 channel-major [320, E] and unpacked on host.

Self-contained: hardcodes shapes from the problem spec; no file reads.
"""
import os
import sys

import numpy as np

# ---------------------------------------------------------------- constants
E_FULL = 200000
N_CORES = 8
E_CORE = E_FULL // N_CORES      # 25000
NT = 512                        # edges per tile
T_TILES = 49                    # tiles per core
E_PAD = NT * T_TILES            # 25088
MUL_S = 128
MUL_V = 64
N_BLK = 6                       # input row-blocks per tile
TILE_COLS = N_BLK * NT          # 3072

INV_S = 1.0 / np.sqrt(MUL_S)
INV_V = 1.0 / np.sqrt(MUL_V)
SQ2 = 1.0 / np.sqrt(2.0)
SQ3 = 1.0 / np.sqrt(3.0)

_REPO_CANDIDATES = (
    "/opt/trn_rl_repo",
    "/root/.axon_site/_ro/trn_rl_repo",
)


def _ensure_repo_on_path():
    try:
        import concourse.bass  # noqa: F401
        return
    except ImportError:
        pass
    for p in _REPO_CANDIDATES:
        if os.path.isdir(p) and p not in sys.path:
            sys.path.insert(0, p)
    import concourse.bass  # noqa: F401


_CACHE = {}


def _build_nc():
    """Build + compile the per-core Bass program (cached)."""
    if "nc" in _CACHE:
        return _CACHE["nc"]
    _ensure_repo_on_path()
    import concourse.mybir as mybir
    import concourse.tile as tile
    from concourse import bacc

    F32 = mybir.dt.float32
    F16 = mybir.dt.float16
    MULT = mybir.AluOpType.mult
    ADD = mybir.AluOpType.add
    AF = mybir.ActivationFunctionType

    nc = bacc.Bacc(trn_type="TRN2", target_bir_lowering=False, debug=False,
                   num_devices=N_CORES)

    # DRAM inputs (per-core shard) ----------------------------------------
    d_in = nc.dram_tensor("in_t", [128, T_TILES * TILE_COLS], F16,
                          kind="ExternalInput")
    d_x2r = nc.dram_tensor("x2r", [4, E_PAD], F16, kind="ExternalInput")
    d_wa0 = nc.dram_tensor("wa0", [128, 128], F16, kind="ExternalInput")
    d_wa1d = nc.dram_tensor("wa1d", [128, 128], F16, kind="ExternalInput")
    d_w2lo = nc.dram_tensor("w2lo", [128, 128], F16, kind="ExternalInput")
    d_w2hi = nc.dram_tensor("w2hi", [128, 128], F16, kind="ExternalInput")
    d_w2v2 = nc.dram_tensor("w2v2", [128, 64], F16, kind="ExternalInput")
    d_wb4s = nc.dram_tensor("wb4s", [128, 128], F16, kind="ExternalInput")
    d_wb5sd = nc.dram_tensor("wb5sd", [128, 128], F16, kind="ExternalInput")
    d_wb4b = nc.dram_tensor("wb4b", [64, 128], F16, kind="ExternalInput")
    d_wb5bd = nc.dram_tensor("wb5bd", [64, 128], F16, kind="ExternalInput")
    d_wcdiag = nc.dram_tensor("wcdiag", [128, 128], F16, kind="ExternalInput")
    d_wc2 = nc.dram_tensor("wc2", [64, 64], F16, kind="ExternalInput")
    d_fc0 = nc.dram_tensor("fc0", [128, 64], F16, kind="ExternalInput")
    d_fc1 = nc.dram_tensor("fc1", [64, 64], F16, kind="ExternalInput")
    d_fc2a = nc.dram_tensor("fc2a", [64, 128], F16, kind="ExternalInput")
    d_fc2bd = nc.dram_tensor("fc2bd", [64, 128], F16, kind="ExternalInput")
    d_b0 = nc.dram_tensor("b0c", [64, 1], F32, kind="ExternalInput")
    d_b1 = nc.dram_tensor("b1c", [64, 1], F32, kind="ExternalInput")
    d_b2a = nc.dram_tensor("b2a", [128, 1], F32, kind="ExternalInput")
    d_b2bh = nc.dram_tensor("b2bh", [128, 1], F32, kind="ExternalInput")

    d_out = nc.dram_tensor("out_t", [320, E_PAD], F16, kind="ExternalOutput")

    with tile.TileContext(nc) as tc:
        with tc.tile_pool(name="const", bufs=1) as cp, \
             tc.tile_pool(name="io", bufs=3) as io, \
             tc.tile_pool(name="rep", bufs=3) as rp, \
             tc.tile_pool(name="work", bufs=2) as wk, \
             tc.tile_pool(name="ps", bufs=1, space="PSUM") as ps:

            def const(d, shape, dtype=F16):
                t = cp.tile(shape, dtype, name=d.name + "_sb")
                nc.sync.dma_start(t, d.ap())
                return t

            w_wa0 = const(d_wa0, [128, 128])
            w_wa1d = const(d_wa1d, [128, 128])
            w_w2lo = const(d_w2lo, [128, 128])
            w_w2hi = const(d_w2hi, [128, 128])
            w_w2v2 = const(d_w2v2, [128, 64])
            w_wb4s = const(d_wb4s, [128, 128])
            w_wb5sd = const(d_wb5sd, [128, 128])
            w_wb4b = const(d_wb4b, [64, 128])
            w_wb5bd = const(d_wb5bd, [64, 128])
            w_wcdiag = const(d_wcdiag, [128, 128])
            w_fc0 = const(d_fc0, [128, 64])
            w_fc1 = const(d_fc1, [64, 64])
            w_fc2a = const(d_fc2a, [64, 128])
            w_fc2bd = const(d_fc2bd, [64, 128])
            c_b0 = const(d_b0, [64, 1], F32)
            c_b1 = const(d_b1, [64, 1], F32)
            c_b2a = const(d_b2a, [128, 1], F32)
            c_b2bh = const(d_b2bh, [128, 1], F32)
            # w3f copy at partitions 64-127 for the row-offset matmul
            w_wc_f = cp.tile([128, 64], F16, name="wc_hi_sb")
            w_wc_hi = w_wc_f[64:128, :]
            nc.sync.dma_start(w_wc_hi, d_wc2.ap())

            # per-tile state rings, keyed by tile index
            S = {}

            def load(k):
                csl = slice(k * TILE_COLS, (k + 1) * TILE_COLS)
                mega = io.tile([128, TILE_COLS], F16)
                nc.sync.dma_start(mega, d_in.ap()[:, csl])
                # one broadcast DMA: [s|v0|v1|v2] full-height replication
                rfull = rp.tile([128, 4 * NT], F16)
                src = (d_x2r.ap()[:, k * NT:(k + 1) * NT]
                       .unsqueeze(0).broadcast_to([128, 4, NT]))
                nc.sync.dma_start(rfull, src)
                S[k] = {"mega": mega, "rfull": rfull}

            def mega_slices(k):
                m = S[k]["mega"]
                r = S[k]["rfull"]
                return {
                    "x1s": m[:, 0 * NT:1 * NT],
                    "x1va": m[:, 1 * NT:2 * NT],
                    "x2d": m[:, 2 * NT:3 * NT],
                    "fwt": m[:, 3 * NT:4 * NT],
                    "r_v01": m[:, 4 * NT:5 * NT],
                    "r_sv2": m[:, 5 * NT:6 * NT],
                    "r_s": r[:, 0 * NT:1 * NT],
                    "r_v0": r[:, 1 * NT:2 * NT],
                    "r_v1": r[:, 2 * NT:3 * NT],
                    "r_v2": r[:, 3 * NT:4 * NT],
                }

            def prep_dve_a(k):
                sk = S[k]
                ms = mega_slices(k)
                x1s_s = wk.tile([128, NT], F16)
                nc.vector.tensor_tensor(x1s_s, ms["x1s"], ms["r_s"], MULT)
                xv_s01 = wk.tile([128, NT], F16)
                nc.vector.tensor_tensor(xv_s01, ms["x1va"], ms["r_s"], MULT)
                xv_p01 = wk.tile([128, NT], F16)
                nc.vector.tensor_tensor(xv_p01, ms["x1va"], ms["r_v01"], MULT)
                x1s_v0 = wk.tile([128, NT], F16)
                nc.vector.tensor_tensor(x1s_v0, ms["x1s"], ms["r_v0"], MULT)
                sk.update(x1s_s=x1s_s, xv_s01=xv_s01, xv_p01=xv_p01,
                          x1s_v0=x1s_v0)

            def prep_dve_b(k):
                sk = S[k]
                ms = mega_slices(k)
                x1s_v1 = wk.tile([128, NT], F16)
                nc.vector.tensor_tensor(x1s_v1, ms["x1s"], ms["r_v1"], MULT)
                x1s_v2 = wk.tile([128, NT], F16)
                nc.vector.tensor_tensor(x1s_v2, ms["x1s"], ms["r_v2"], MULT)
                xv_sp2 = wk.tile([128, NT], F16)
                nc.gpsimd.tensor_tensor(xv_sp2, ms["x2d"], ms["r_sv2"], MULT)
                sk.update(x1s_v1=x1s_v1, x1s_v2=x1s_v2, xv_sp2=xv_sp2)

            def prep(k):
                """MLP front (h1) for tile k."""
                sk = S[k]
                ms = mega_slices(k)
                h1 = ps.tile([64, NT], F32, tag="h12")
                nc.tensor.matmul(h1, w_fc0, ms["fwt"], start=True, stop=True)
                h1s = wk.tile([64, NT], F16)
                nc.scalar.activation(h1s, h1, AF.Silu, bias=c_b0)
                sk.update(h1s=h1s)

            def prep_h2(k):
                sk = S[k]
                h2 = ps.tile([64, NT], F32, tag="h12")
                nc.tensor.matmul(h2, w_fc1, sk["h1s"], start=True, stop=True)
                h2s = wk.tile([64, NT], F16)
                nc.scalar.activation(h2s, h2, AF.Silu, bias=c_b1)
                sk.update(h2s=h2s)

            def main_mlp2(k):
                """wwa / wwb matmuls + crossings for tile k."""
                sk = S[k]
                wwa = ps.tile([128, NT], F32, tag="wwa")
                nc.tensor.matmul(wwa, w_fc2a, sk["h2s"], start=True, stop=True)
                wwb = ps.tile([128, NT], F32, tag="wwb")
                nc.tensor.matmul(wwb, w_fc2bd, sk["h2s"],
                                 start=True, stop=True)
                wbs = wk.tile([128, NT], F16)
                nc.scalar.activation(wbs, wwb, AF.Identity, bias=c_b2bh)
                was = wk.tile([128, NT], F16)
                nc.scalar.activation(was, wwa, AF.Identity, bias=c_b2a)
                sk.update(wbs=wbs, was=was)

            def main_tp_a(k):
                """Six accumulating TP matmuls for tile k."""
                sk = S[k]
                scal = ps.tile([128, NT], F32, tag="scal")
                gate2 = ps.tile([128, NT], F32, tag="gate2")
                nc.tensor.matmul(scal, w_wa0, sk["x1s_s"],
                                 start=True, stop=False)
                nc.tensor.matmul(gate2, w_wa1d, sk["x1s_s"],
                                 start=True, stop=False)
                nc.tensor.matmul(scal, w_wb4s, sk["xv_p01"],
                                 start=False, stop=False)
                nc.tensor.matmul(gate2, w_wb5sd, sk["xv_p01"],
                                 start=False, stop=False)
                nc.tensor.matmul(scal, w_wb4b, sk["xv_sp2"][0:64, :],
                                 start=False, stop=True)
                nc.tensor.matmul(gate2, w_wb5bd, sk["xv_sp2"][0:64, :],
                                 start=False, stop=True)
                sk.update(scal=scal, gate2=gate2)

            def main_vec(k):
                """vec01 / vec2 accumulation groups."""
                sk = S[k]
                vec01 = ps.tile([128, NT], F32, tag="vec01", bufs=2)
                vec2 = ps.tile([64, NT], F32, tag="vec2")
                nc.tensor.matmul(vec01, w_wcdiag, sk["xv_s01"],
                                 start=True, stop=False)
                nc.tensor.matmul(vec01, w_w2lo, sk["x1s_v0"],
                                 start=False, stop=False)
                nc.tensor.matmul(vec01, w_w2hi, sk["x1s_v1"],
                                 start=False, stop=True)
                nc.tensor.matmul(vec2, w_w2v2, sk["x1s_v2"],
                                 start=True, stop=False)
                nc.tensor.matmul(vec2, w_wc_hi, sk["xv_sp2"][64:128, :],
                                 start=False, stop=True,
                                 tile_position=(64, 0))
                sk.update(vec01=vec01, vec2=vec2)

            def main_act(k):
                sk = S[k]
                sc_silu = wk.tile([128, NT], F16)
                nc.scalar.activation(sc_silu, sk["scal"], AF.Silu)
                tg = wk.tile([128, NT], F16)
                nc.scalar.activation(tg, sk["gate2"], AF.Tanh, scale=0.5)
                sk.update(sc_silu=sc_silu, tg=tg)

            def out_phase(k):
                """Sigmoid chain + output muls + stores for tile k."""
                sk = S[k]
                sl = slice(k * NT, (k + 1) * NT)
                # sgw2 = (tg+1)*wbs = tg*wbs + wbs
                sgu = wk.tile([128, NT], F16)
                nc.gpsimd.tensor_tensor(sgu, sk["tg"], sk["wbs"], MULT)
                sgw2 = wk.tile([128, NT], F16)
                nc.vector.tensor_tensor(sgw2, sgu, sk["wbs"], ADD)
                out01 = wk.tile([128, NT], F16)
                nc.vector.tensor_tensor(out01, sk["vec01"], sgw2, MULT)
                out2 = wk.tile([64, NT], F16)
                nc.vector.tensor_tensor(out2, sk["vec2"], sgw2[0:64, :], MULT)
                out_s = wk.tile([128, NT], F16)
                nc.vector.tensor_tensor(out_s, sk["sc_silu"], sk["was"], MULT)
                nc.sync.dma_start(d_out.ap()[128:256, sl], out01)
                nc.gpsimd.dma_start(d_out.ap()[256:320, sl], out2)
                nc.sync.dma_start(d_out.ap()[0:128, sl], out_s)

            # ---- pipelined emission --------------------------------
            # iteration k emits: load(k+1) | prep(k) | main(k-1) | out(k-2)
            T = T_TILES
            for k in range(T + 2):
                if k == 0:
                    load(0)
                if k + 1 < T:
                    load(k + 1)
                m = k - 1        # main tile
                o = k - 2        # output tile
                if k < T:
                    prep_dve_a(k)
                if 0 <= o < T:
                    out_phase(o)
                if k < T:
                    prep_dve_b(k)
                    prep(k)           # PE 1: h1; ACT h1s
                if 0 <= m < T:
                    main_mlp2(m)      # PE 2-3; ACT wbs, was
                    main_tp_a(m)      # PE 4-9
                if k < T:
                    prep_h2(k)        # PE 10; ACT h2s
                if 0 <= m < T:
                    main_vec(m)       # PE 11-15
                    main_act(m)       # ACT sc_silu, tg
                if o - 1 in S:
                    del S[o - 1]

    nc.compile()
    _CACHE["nc"] = nc
    return nc


def _fold_weights(inp):
    """Fold per-channel weights + constants into fp16 matmul matrices."""
    f = lambda k: np.asarray(inp[k], dtype=np.float32)
    w0f = f("w1_p0") * f("w2_p0")[None, :] * (INV_S * SQ2)
    w1f = f("w1_p1") * f("w2_p1")[None, :] * (INV_S * SQ2)
    w2f = f("w1_p2") * f("w2_p2")[None, :] * (INV_S * SQ2)
    w3f = f("w1_p3") * f("w2_p3")[None, :] * (INV_V * SQ2)
    w4f = f("w1_p4") * f("w2_p4")[None, :] * (INV_V * SQ3 * SQ2)
    w5f = f("w1_p5") * f("w2_p5")[None, :] * (INV_V * SQ3 * SQ2)
    fc2 = f("fc_w2")
    b2 = f("fc_b2")
    w5d = np.concatenate([w5f, w5f], axis=1)         # [64,128] col-dup
    cdiag = np.zeros((128, 128), np.float32)
    cdiag[0:64, 0:64] = w3f
    cdiag[64:128, 64:128] = w3f
    z64 = np.zeros((128, 64), np.float32)
    h = lambda a: np.ascontiguousarray(a.astype(np.float16))
    c = lambda a: np.ascontiguousarray(a.astype(np.float32))
    return {
        "wa0": h(w0f),
        "wa1d": h(np.concatenate([w1f, w1f], axis=1)),
        "w2lo": h(np.concatenate([w2f, z64], axis=1)),
        "w2hi": h(np.concatenate([z64, w2f], axis=1)),
        "w2v2": h(w2f),
        "wb4s": h(np.concatenate([w4f, w4f], axis=0)),
        "wb5sd": h(np.concatenate([w5d, w5d], axis=0)),
        "wb4b": h(w4f),
        "wb5bd": h(w5d),
        "wcdiag": h(cdiag),
        "wc2": h(w3f),
        "fc0": h(f("fc_w0")),
        "fc1": h(f("fc_w1")),
        "fc2a": h(fc2[:, :128]),
        "fc2bd": h(0.5 * np.concatenate([fc2[:, 128:], fc2[:, 128:]],
                                        axis=1)),
        "b0c": c(f("fc_b0")[:, None]),
        "b1c": c(f("fc_b1")[:, None]),
        "b2a": c(b2[:128, None]),
        "b2bh": c(0.5 * np.concatenate([b2[128:], b2[128:]])[:, None]),
    }


def _shard_inputs(inp):
    """Per-core merged fp16 input tensor + tiny x2 scalar rows."""
    fea_in1 = np.asarray(inp["fea_in1"], dtype=np.float32)
    fea_in2 = np.asarray(inp["fea_in2"], dtype=np.float32)
    fea_w = np.asarray(inp["fea_weight"], dtype=np.float32)
    shards = []
    for cidx in range(N_CORES):
        s = slice(cidx * E_CORE, (cidx + 1) * E_CORE)
        x1 = fea_in1[s]
        x2 = fea_in2[s]
        fw = fea_w[s]
        blocks = np.zeros((N_BLK, 128, E_PAD), np.float16)
        blocks[0][:, :E_CORE] = x1[:, :128].T.astype(np.float16)
        xv = x1[:, 128:].reshape(E_CORE, 64, 3).transpose(2, 1, 0)  # [3,64,E]
        blocks[1][0:64, :E_CORE] = xv[0].astype(np.float16)
        blocks[1][64:128, :E_CORE] = xv[1].astype(np.float16)
        blocks[2][0:64, :E_CORE] = xv[2].astype(np.float16)
        blocks[2][64:128, :E_CORE] = xv[2].astype(np.float16)
        blocks[3][:, :E_CORE] = fw.T.astype(np.float16)
        # x2 rows: [s, v0, v1, v2]
        x2p = np.zeros((4, E_PAD), np.float16)
        x2p[0, :E_CORE] = x2[:, 0].astype(np.float16)
        x2p[1, :E_CORE] = x2[:, 1].astype(np.float16)
        x2p[2, :E_CORE] = x2[:, 2].astype(np.float16)
        x2p[3, :E_CORE] = x2[:, 3].astype(np.float16)
        blocks[4][0:64, :] = x2p[1][None, :]      # r_v01 = [v0; v1]
        blocks[4][64:128, :] = x2p[2][None, :]
        blocks[5][0:64, :] = x2p[3][None, :]      # r_sv2 = [v2; s]
        blocks[5][64:128, :] = x2p[0][None, :]
        mega = (blocks.reshape(N_BLK, 128, T_TILES, NT)
                .transpose(1, 2, 0, 3)
                .reshape(128, T_TILES * TILE_COLS))
        shards.append({"in_t": np.ascontiguousarray(mega),
                       "x2r": np.ascontiguousarray(x2p)})
    return shards


def run(inputs, trace=False, trace_kwargs=None):
    """Run the kernel; returns (output [E,320] f32, BassKernelResults)."""
    _ensure_repo_on_path()
    from concourse import bass_utils

    nc = _build_nc()
    weights = _fold_weights(inputs)
    shards = _shard_inputs(inputs)
    in_maps = [{**weights, **sh} for sh in shards]

    kwargs = {}
    if trace:
        _install_ntff_hook()
        kwargs.update(trace=True, **(trace_kwargs or {}))
    res = bass_utils.run_bass_kernel_spmd(
        nc, in_maps, core_ids=list(range(N_CORES)), **kwargs)

    out = np.empty((E_FULL, 320), np.float32)
    for c in range(N_CORES):
        o = res.results[c]["out_t"][:, :E_CORE].astype(np.float32)
        s = slice(c * E_CORE, (c + 1) * E_CORE)
        out[s, :128] = o[:128].T
        out[s, 128:] = (o[128:].reshape(3, 64, E_CORE)
                        .transpose(2, 1, 0).reshape(E_CORE, 192))
    return out, res


def _install_ntff_hook():
    """Shim the missing antenv.axon_hooks so trace=True works under axon."""
    import types
    import antenv
    from concourse import bass_utils
    if "antenv.axon_hooks" in sys.modules:
        return
    mod = types.ModuleType("antenv.axon_hooks")
    _h = [None]
    mod.set_axon_ntff_profile_hook = lambda h: _h.__setitem__(0, h)
    mod.get_axon_ntff_profile_hook = lambda: _h[0]
    sys.modules["antenv.axon_hooks"] = mod
    antenv.axon_hooks = mod
    from trn_agent_boot.trn_boot import _ntff_profile_via_ctypes
    mod.set_axon_ntff_profile_hook(
        _ntff_profile_via_ctypes("/opt/axon/libaxon_pjrt.so"))
    bass_utils.upload_artifacts = lambda tmpdir: tmpdir


def kernel(**inputs) -> np.ndarray:
    out, _ = run(inputs, trace=False)
    return out
